# revision 1
# baseline (speedup 1.0000x reference)
"""Trainium2 Bass kernel for AttentionConstrainedLoss.

Contract: kernel(atten_map [16,1600,2048] f32, gt_bboxes [16,64,7] f32) -> scalar f32.

Strategy (data-parallel over batch, 2 scenes per core on 8 cores):
  - per cell: variance over the 2048 feature dim (memory-bound part, ~26 MB/core)
  - per scene: box->grid assignment. The reference's sequential overwrite
    rule has a closed form: flag[g] = (#covering boxes odd) ? max covering
    box index : -1, which vectorizes fully.
  - segment mean of variance by flag via onehot matmuls on the PE.
  - per-core partial [sum(means), sum(counts>0)]; final scalar combined on host.
"""

from contextlib import ExitStack

import numpy as np

_CACHE = {}

# problem constants (hardcoded per spec)
B, G, D, M = 16, 1600, 2048, 64
NCORES = 8
BPC = B // NCORES          # batches per core = 2
ROWS = BPC * G             # 3200 rows of [D] per core
NCH = 13                   # 13 chunks of <=128 rows per batch (12*128 + 64)
NCOL = BPC * NCH           # 26 stat columns per core
# chunks alternate between the DVE bn_stats path (even c) and the ACT
# accumulate path (odd c) so both engines stream throughout the DMA stream

F2 = float(np.float64(102.4) / np.float64(40.0))   # 2.56 as python float


def _build_program(dma_engines=("sync",), reps=1, pair_dma=False):
    import concourse.bacc as bacc
    import concourse.tile as tile
    from concourse import mybir

    f32 = mybir.dt.float32
    i32 = mybir.dt.int32
    op = mybir.AluOpType
    AF = mybir.ActivationFunctionType
    X = mybir.AxisListType.X

    nc = bacc.Bacc("TRN2", target_bir_lowering=False, debug=False,
                   enable_asserts=True, num_devices=NCORES)

    x_d = nc.declare_dram_parameter("x", [ROWS, D], f32, isOutput=False)
    bb_d = nc.declare_dram_parameter("bb", [2 * M, 7], f32, isOutput=False)
    out_d = nc.declare_dram_parameter("out", [2, 1], f32, isOutput=True)

    with tile.TileContext(nc) as tc, ExitStack() as ctx:
        singles = ctx.enter_context(tc.tile_pool(name="singles", bufs=1))
        xpool = ctx.enter_context(tc.tile_pool(name="x", bufs=1))
        bnpool = ctx.enter_context(tc.tile_pool(name="bn", bufs=3))
        ohpool = ctx.enter_context(tc.tile_pool(name="oh", bufs=3))
        tpps = ctx.enter_context(tc.tile_pool(name="tpps", bufs=3, space="PSUM"))
        segps = ctx.enter_context(tc.tile_pool(name="segps", bufs=2, space="PSUM"))
        finps = ctx.enter_context(tc.tile_pool(name="finps", bufs=1, space="PSUM"))

        # ---------------- constants (generated on device) ----------------
        # grid coords px/py replicated on all 128 partitions: [128, G]
        # iota values are small integers -> exact in f32
        px = singles.tile([128, G], f32)
        nc.gpsimd.iota(px, pattern=[[0, 40], [1, 40]], base=0,
                       channel_multiplier=0, allow_small_or_imprecise_dtypes=True)
        py = singles.tile([128, G], f32)
        nc.gpsimd.iota(py, pattern=[[1, 40], [0, 40]], base=0,
                       channel_multiplier=0, allow_small_or_imprecise_dtypes=True)
        # (w + 0.5) * (1/40) * 102.4 - 51.2, each step f32 (DVE has no divide;
        # 1-ulp grid deviations verified to not change the loss)
        r40 = float(np.float32(1.0) / np.float32(40.0))
        for gtile in (px, py):
            nc.vector.tensor_scalar(out=gtile, in0=gtile, scalar1=0.5,
                                    scalar2=r40, op0=op.add, op1=op.mult)
            nc.vector.tensor_scalar(out=gtile, in0=gtile,
                                    scalar1=float(np.float32(102.4)),
                                    scalar2=float(np.float32(-51.2)),
                                    op0=op.mult, op1=op.add)

        # onehot comparison row: 0..63 on every partition, f32
        iota64f = singles.tile([128, M], f32)
        nc.gpsimd.iota(iota64f, pattern=[[1, M]], base=0, channel_multiplier=0,
                       allow_small_or_imprecise_dtypes=True)

        # weight row for "last covering box": (j % 64) + 1 over [13, 128] blocks
        wrow = singles.tile([128, NCH, 128], f32)
        nc.gpsimd.iota(wrow, pattern=[[0, NCH], [0, 2], [1, M]], base=1,
                       channel_multiplier=0, allow_small_or_imprecise_dtypes=True)

        # identity for PE transposes: (i - p) == 0
        ident = singles.tile([128, 128], f32)
        nc.gpsimd.iota(ident, pattern=[[1, 128]], base=0, channel_multiplier=-1,
                       allow_small_or_imprecise_dtypes=True)
        nc.vector.tensor_scalar(out=ident, in0=ident, scalar1=0.0,
                                scalar2=None, op0=op.is_equal)

        ones64 = singles.tile([64, 1], f32)
        nc.vector.memset(ones64, 1.0)

        # ---------------- per-box scalars (boxes of both scenes on partitions) --
        bb = singles.tile([128, 7], f32)
        nc.sync.dma_start(out=bb, in_=bb_d.ap())
        cx, cy = bb[:, 0:1], bb[:, 1:2]
        bl, bw = bb[:, 3:4], bb[:, 4:5]
        yaw = bb[:, 6:7]

        # ratio = clip(2.56 / dim, 1, 6) via reciprocal (no divide ALU op)
        ratl = singles.tile([128, 1], f32)
        nc.vector.reciprocal(ratl, bl)
        nc.vector.tensor_scalar(out=ratl, in0=ratl, scalar1=F2, scalar2=1.0,
                                op0=op.mult, op1=op.max)
        nc.vector.tensor_scalar(out=ratl, in0=ratl, scalar1=6.0, scalar2=None,
                                op0=op.min)
        ratw = singles.tile([128, 1], f32)
        nc.vector.reciprocal(ratw, bw)
        nc.vector.tensor_scalar(out=ratw, in0=ratw, scalar1=F2, scalar2=1.0,
                                op0=op.mult, op1=op.max)
        nc.vector.tensor_scalar(out=ratw, in0=ratw, scalar1=6.0, scalar2=None,
                                op0=op.min)
        el = singles.tile([128, 1], f32)
        nc.vector.tensor_tensor(out=el, in0=bl, in1=ratl, op=op.mult)
        ew = singles.tile([128, 1], f32)
        nc.vector.tensor_tensor(out=ew, in0=bw, in1=ratw, op=op.mult)

        sin_t = singles.tile([128, 1], f32)
        cos_t = singles.tile([128, 1], f32)
        halfpi = singles.tile([128, 1], f32)
        nc.vector.memset(halfpi, float(np.pi / 2))
        nc.scalar.activation(sin_t, yaw, AF.Sin)
        # cos(x) = cos(|x|) = sin(pi/2 - |x|), keeps the Sin arg in [-pi, pi]
        absyaw = singles.tile([128, 1], f32)
        nc.scalar.activation(absyaw, yaw, AF.Abs)
        nc.scalar.activation(cos_t, absyaw, AF.Sin, bias=halfpi[:, 0:1],
                             scale=-1.0)

        sw = singles.tile([128, 1], f32)
        nc.vector.tensor_tensor(out=sw, in0=sin_t, in1=ew, op=op.mult)
        cw = singles.tile([128, 1], f32)
        nc.vector.tensor_tensor(out=cw, in0=cos_t, in1=ew, op=op.mult)
        cl = singles.tile([128, 1], f32)
        nc.vector.tensor_tensor(out=cl, in0=cos_t, in1=el, op=op.mult)
        sl = singles.tile([128, 1], f32)
        nc.vector.tensor_tensor(out=sl, in0=sin_t, in1=el, op=op.mult)

        # midS = cw*cx + sw*cy ; midTn = sl*cx - cl*cy ; half = el*ew/2
        t1 = singles.tile([128, 1], f32)
        t2 = singles.tile([128, 1], f32)
        nc.vector.tensor_tensor(out=t1, in0=cw, in1=cx, op=op.mult)
        nc.vector.tensor_tensor(out=t2, in0=sw, in1=cy, op=op.mult)
        midS = singles.tile([128, 1], f32)
        nc.vector.tensor_tensor(out=midS, in0=t1, in1=t2, op=op.add)
        nc.vector.tensor_tensor(out=t1, in0=sl, in1=cx, op=op.mult)
        nc.vector.tensor_tensor(out=t2, in0=cl, in1=cy, op=op.mult)
        midTn = singles.tile([128, 1], f32)
        nc.vector.tensor_tensor(out=midTn, in0=t1, in1=t2, op=op.subtract)
        half = singles.tile([128, 1], f32)
        nc.vector.tensor_tensor(out=half, in0=el, in1=ew, op=op.mult)
        nc.vector.tensor_scalar(out=half, in0=half, scalar1=0.5, scalar2=None,
                                op0=op.mult)
        # range bounds: [mid - half, mid + half] per axis (no abs op in TS ISA)
        lo0 = singles.tile([128, 1], f32)
        hi0 = singles.tile([128, 1], f32)
        nc.vector.tensor_tensor(out=lo0, in0=midS, in1=half, op=op.subtract)
        nc.vector.tensor_tensor(out=hi0, in0=midS, in1=half, op=op.add)
        lo1 = singles.tile([128, 1], f32)
        hi1 = singles.tile([128, 1], f32)
        nc.vector.tensor_tensor(out=lo1, in0=midTn, in1=half, op=op.subtract)
        nc.vector.tensor_tensor(out=hi1, in0=midTn, in1=half, op=op.add)

        # ------------- inside masks, boxes-on-partitions layout [128, G] --------
        # box-frame coords scaled by area: S = cw*px + sw*py, Tn = sl*px - cl*py
        # inside <=> |S - midS| <= half and |Tn - midTn| <= half
        s_t = singles.tile([128, G], f32)
        nc.vector.tensor_scalar(out=s_t, in0=py, scalar1=sw[:, 0:1],
                                scalar2=None, op0=op.mult)
        nc.vector.scalar_tensor_tensor(out=s_t, in0=px, scalar=cw[:, 0:1],
                                       in1=s_t, op0=op.mult, op1=op.add)
        va = singles.tile([128, G], f32)
        nc.vector.tensor_scalar(out=va, in0=s_t, scalar1=lo0[:, 0:1],
                                scalar2=None, op0=op.is_ge)
        nc.vector.tensor_scalar(out=s_t, in0=s_t, scalar1=hi0[:, 0:1],
                                scalar2=None, op0=op.is_le)
        nc.gpsimd.tensor_tensor(out=s_t, in0=s_t, in1=va, op=op.mult)

        tt_t = singles.tile([128, G], f32)
        nc.vector.tensor_scalar(out=tt_t, in0=py, scalar1=cl[:, 0:1],
                                scalar2=None, op0=op.mult)
        nc.vector.scalar_tensor_tensor(out=tt_t, in0=px, scalar=sl[:, 0:1],
                                       in1=tt_t, op0=op.mult, op1=op.subtract)
        nc.vector.tensor_scalar(out=va, in0=tt_t, scalar1=lo1[:, 0:1],
                                scalar2=None, op0=op.is_ge)
        nc.vector.tensor_scalar(out=tt_t, in0=tt_t, scalar1=hi1[:, 0:1],
                                scalar2=None, op0=op.is_le)
        nc.gpsimd.tensor_tensor(out=tt_t, in0=tt_t, in1=va, op=op.mult)

        inside = tt_t
        nc.gpsimd.tensor_tensor(out=inside, in0=s_t, in1=tt_t, op=op.mult)

        # nearest grid cell to each box center is always included
        dx_t = singles.tile([128, G], f32)
        nc.vector.tensor_scalar(out=dx_t, in0=px, scalar1=cx, scalar2=None,
                                op0=op.subtract)
        dy_t = singles.tile([128, G], f32)
        nc.vector.tensor_scalar(out=dy_t, in0=py, scalar1=cy, scalar2=None,
                                op0=op.subtract)
        nc.scalar.activation(dx_t, dx_t, AF.Square)
        nc.gpsimd.tensor_tensor(out=dy_t, in0=dy_t, in1=dy_t, op=op.mult)
        dist = dx_t
        nc.gpsimd.tensor_tensor(out=dist, in0=dx_t, in1=dy_t, op=op.add)
        mind = singles.tile([128, 1], f32)
        nc.vector.tensor_reduce(out=mind, in_=dist, axis=X, op=op.min)
        near = dy_t
        nc.vector.tensor_scalar(out=near, in0=dist, scalar1=mind[:, 0:1],
                                scalar2=None, op0=op.is_equal)
        mask = singles.tile([128, G], f32)
        nc.vector.tensor_tensor(out=mask, in0=inside, in1=near, op=op.max)

        # ------------- transpose masks to cells-on-partitions -------------------
        # mask_T[p, t, j]: cell t*128+p, box j%64 of scene j//64
        mask_T = singles.tile([128, NCH, 128], f32)
        nc.vector.memset(mask_T[:, NCH - 1:NCH, :], 0.0)
        for t in range(NCH):
            csz = 128 if t < NCH - 1 else G - 128 * (NCH - 1)
            ps = tpps.tile([128, 128], f32, tag="tp")
            nc.tensor.transpose(ps[:csz, :], mask[:, t * 128:t * 128 + csz], ident)
            if t % 2 == 0:
                nc.vector.tensor_copy(mask_T[:csz, t:t + 1, :], ps[:csz, :])
            else:
                nc.scalar.copy(mask_T[:csz, t:t + 1, :], ps[:csz, :])

        wmask = singles.tile([128, NCH, 128], f32)
        nc.gpsimd.tensor_tensor(out=wmask, in0=mask_T, in1=wrow, op=op.mult)

        # ------------- flags per scene: [128, 13] -------------------------------
        flags = []
        for b in range(BPC):
            cnt = singles.tile([128, NCH], f32, tag=f"cnt{b}")
            nc.vector.tensor_reduce(out=cnt, in_=mask_T[:, :, b * M:(b + 1) * M],
                                    axis=X, op=op.add)
            wmx = singles.tile([128, NCH], f32, tag=f"wmx{b}")
            nc.vector.tensor_reduce(out=wmx, in_=wmask[:, :, b * M:(b + 1) * M],
                                    axis=X, op=op.max)
            # parity of integer-valued cnt: h = cnt/2; r = round-half-even(h)
            # (add/sub 2^23); odd = 2*|h - r|
            h = singles.tile([128, NCH], f32, tag=f"h{b}")
            nc.vector.tensor_scalar(out=h, in0=cnt, scalar1=0.5, scalar2=None,
                                    op0=op.mult)
            r = singles.tile([128, NCH], f32, tag=f"r{b}")
            nc.vector.tensor_scalar(out=r, in0=h, scalar1=8388608.0,
                                    scalar2=8388608.0, op0=op.add,
                                    op1=op.subtract)
            odd = singles.tile([128, NCH], f32, tag=f"odd{b}")
            nc.vector.tensor_tensor(out=odd, in0=h, in1=r, op=op.subtract)
            nc.scalar.activation(odd, odd, AF.Abs, scale=2.0)
            flag = singles.tile([128, NCH], f32, tag=f"flag{b}")
            nc.vector.tensor_tensor(out=flag, in0=odd, in1=wmx, op=op.mult)
            nc.vector.tensor_scalar(out=flag, in0=flag, scalar1=1.0,
                                    scalar2=None, op0=op.subtract)
            flags.append(flag)

        # ------------- variance over feature dim + per-chunk segment matmul -----
        # stats[p, pair, parity, 0:2]: column c=(2*pair+parity);
        # parity 0 (bn path) -> (mean, var_pop); parity 1 (act path) -> (sum, sumsq)
        # Each chunk's variance column and onehot matmul happen as soon as its
        # stats land, so only the last chunk's work sits in the kernel tail.
        stats = singles.tile([128, NCH, 2, 2], f32)
        nc.vector.memset(stats, 0.0)
        vrhs = singles.tile([128, NCH, 2, 2], f32)
        nc.vector.memset(vrhs, 1.0)
        segs = [segps.tile([M, 2], f32, tag=f"seg{b}", name=f"seg{b}")
                for b in range(BPC)]
        K1 = float(np.float32(D / (D - 1.0)))
        K2 = float(np.float32(-1.0 / (2047.0 * 2048.0)))
        K3 = float(np.float32(1.0 / 2047.0))
        xap = x_d.ap()

        def chunk_dma(b, t, c):
            """DMA one chunk, return its SBUF tile view [csz, D]."""
            r0 = b * G + t * 128
            csz = 128 if t < NCH - 1 else G - 128 * (NCH - 1)
            eng = getattr(nc, dma_engines[c % len(dma_engines)])
            if pair_dma and t % 2 == 0 and t + 1 < NCH - 1:
                # one 2 MB DMA covering chunks t and t+1
                xt2 = xpool.tile([128, 2, D], f32, tag="xt2", name="xt2", bufs=4)
                src = xap[r0:r0 + 256, :].rearrange("(two p) d -> p two d", p=128)
                eng.dma_start(out=xt2, in_=src)
                chunk_dma.pending = xt2
                return xt2[:, 0, :]
            if pair_dma and t % 2 == 1 and t < NCH - 1:
                return chunk_dma.pending[:, 1, :]
            xt = xpool.tile([128, D], f32, tag="xt", name="xt",
                            bufs=2 if pair_dma else 8)
            split = 4 if c == NCOL - 1 else 1
            for j in range(split):
                w = D // split
                eng.dma_start(out=xt[:csz, j * w:(j + 1) * w],
                              in_=xap[r0:r0 + csz, j * w:(j + 1) * w])
            return xt[:csz, :]

        def variance_pass():
          for b in range(BPC):
            for t in range(NCH):
                c = b * NCH + t
                # storage slot (pair, parity); ACT path on even chunks, bn (DVE)
                # on odd ones, so the final chunk is bn (shortest post-DMA
                # latency) and both engines stream throughout
                pr, q = c // 2, c % 2
                use_act = (c % 2 == 0)
                csz = 128 if t < NCH - 1 else G - 128 * (NCH - 1)
                xt = chunk_dma(b, t, c)
                if not use_act:
                    st = bnpool.tile([128, 4, 6], f32, tag="bnst")
                    for j in range(4):
                        nc.vector.bn_stats(out=st[:csz, j:j + 1, :],
                                           in_=xt[:csz, j * 512:(j + 1) * 512])
                    nc.vector.bn_aggr(out=stats[:csz, pr, q, :], in_=st[:csz])
                    nc.vector.tensor_scalar(out=vrhs[:, pr, q, 0:1],
                                            in0=stats[:, pr, q, 1:2],
                                            scalar1=K1, scalar2=None,
                                            op0=op.mult)
                else:
                    nc.scalar.activation(xt[:csz, :], xt[:csz, :], AF.Copy,
                                         accum_out=stats[:csz, pr, q, 0:1])
                    nc.scalar.activation(xt[:csz, :], xt[:csz, :], AF.Square,
                                         accum_out=stats[:csz, pr, q, 1:2])
                    tmp = bnpool.tile([128, 1], f32, tag="vtmp")
                    nc.vector.tensor_tensor(out=tmp, in0=stats[:, pr, q, 0:1],
                                            in1=stats[:, pr, q, 0:1], op=op.mult)
                    nc.vector.tensor_scalar(out=tmp, in0=tmp, scalar1=K2,
                                            scalar2=None, op0=op.mult)
                    nc.vector.scalar_tensor_tensor(out=vrhs[:, pr, q, 0:1],
                                                   in0=stats[:, pr, q, 1:2],
                                                   scalar=K3, in1=tmp,
                                                   op0=op.mult, op1=op.add)
                oh = ohpool.tile([128, M], f32, tag="oh")
                nc.vector.tensor_scalar(out=oh, in0=iota64f,
                                        scalar1=flags[b][:, t:t + 1],
                                        scalar2=None, op0=op.is_equal)
                nc.tensor.matmul(out=segs[b], lhsT=oh, rhs=vrhs[:, pr, q, :],
                                 start=(t == 0), stop=(t == NCH - 1))

        if reps == 1:
            variance_pass()
        else:
            # timing-only variant: repeat the stream; output stays identical
            # (each pass recomputes the same stats and restarts the psum group)
            with tc.For_i(0, reps, 1):
                variance_pass()

        # ------------- per-batch means + final reduction ------------------------
        mv2s = []
        for b in range(BPC):
            seg = segs[b]
            sums = singles.tile([M, 1], f32, tag=f"sums{b}")
            nc.vector.tensor_copy(sums, seg[:, 0:1])
            cntm = singles.tile([M, 1], f32, tag=f"cntm{b}")
            nc.vector.tensor_copy(cntm, seg[:, 1:2])
            mv2 = singles.tile([M, 2], f32, tag=f"mv2{b}")
            # valid = counts > 0 ; mean = sums / max(counts,1) * valid
            nc.vector.tensor_scalar(out=mv2[:, 1:2], in0=cntm, scalar1=0.0,
                                    scalar2=None, op0=op.is_gt)
            c1t = singles.tile([M, 1], f32, tag=f"c1t{b}")
            nc.vector.tensor_scalar(out=c1t, in0=cntm, scalar1=1.0,
                                    scalar2=None, op0=op.max)
            nc.vector.reciprocal(c1t, c1t)
            nc.vector.tensor_tensor(out=mv2[:, 0:1], in0=sums, in1=c1t,
                                    op=op.mult)
            nc.vector.tensor_tensor(out=mv2[:, 0:1], in0=mv2[:, 0:1],
                                    in1=mv2[:, 1:2], op=op.mult)
            mv2s.append(mv2)

        fin = finps.tile([2, 1], f32)
        for b in range(BPC):
            nc.tensor.matmul(out=fin, lhsT=mv2s[b], rhs=ones64,
                             start=(b == 0), stop=(b == BPC - 1))
        fin_sb = singles.tile([2, 1], f32)
        nc.vector.tensor_copy(fin_sb, fin)
        nc.sync.dma_start(out=out_d.ap(), in_=fin_sb)

    nc.compile()
    return nc


DMA_ENGINES = ("sync",)


def _get_program():
    if "nc" not in _CACHE:
        _CACHE["nc"] = _build_program(DMA_ENGINES)
    return _CACHE["nc"]


def _in_maps(atten_map, gt_bboxes):
    atten_map = np.ascontiguousarray(atten_map, dtype=np.float32)
    gt_bboxes = np.ascontiguousarray(gt_bboxes, dtype=np.float32)
    return [
        {
            "x": atten_map[c * BPC:(c + 1) * BPC].reshape(ROWS, D),
            "bb": gt_bboxes[c * BPC:(c + 1) * BPC].reshape(2 * M, 7),
        }
        for c in range(NCORES)
    ]


def _combine(parts):
    total_mean = float(np.sum(parts[:, 0], dtype=np.float64))
    total_valid = float(np.sum(parts[:, 1], dtype=np.float64))
    return np.array(np.float32(-total_mean / max(total_valid, 1.0)))


def _run(atten_map, gt_bboxes, trace=False):
    from concourse.bass_utils import run_bass_kernel_spmd

    nc = _get_program()
    res = run_bass_kernel_spmd(nc, _in_maps(atten_map, gt_bboxes),
                               list(range(NCORES)), trace=trace)
    parts = np.stack([res.results[c]["out"][:, 0] for c in range(NCORES)])
    return _combine(parts), res


def kernel(atten_map, gt_bboxes):
    out, _ = _run(atten_map, gt_bboxes, trace=False)
    return out

